# revision 3
# baseline (speedup 1.0000x reference)
"""Trainium2 Bass kernel for nn_MixAttention (dual-stream attention block).

Sharding: 8 cores = 4 batches x 2 query-halves (data parallel over batch and
sequence). Each core computes K/V projections for its full batch (duplicated
across the 2 cores sharing a batch) and Q projections + attention + output
projection + layernorm for its own 1024 query rows. No collectives needed.

Math per core (Sq=1024 own query rows, Sk=2048 keys of own batch, H=8, DH=64):
  qcat_h = [qd_h; qt_h], kcat_h = [kd_h; kt_h]  (stream-cat on the partition
       axis -> both dual-stream score terms fuse into one K=128 matmul)
  scoresT_h[t,s] = sum_c kcat_h[c,t] qcat_h[c,s]   (computed transposed so
       the PV matmul needs no transposes of the attention matrix)
  attT_h = exp(scoresT_h / 8 - C) in fp8e4m3, C=5.5 keeps the max under the
       e4m3 ceiling; the shift cancels in the softmax normalization. The exp
       is split between the ACT engine (hw Exp) and DVE (Schraudolph bit-cast
       exp: u8 = round(s*log2e + bias), reinterpreted as e4m3).
  ctxU_h = vsum8_h^T @ attT_h via fp8 DoubleRow matmuls (K=256/step, 2x rate);
       a ones column woven into vsum8 col 64 yields softmax sums in psum row
       64 for free.
  ctx_h = ctxU_h * (1/r); head pairs are stacked on partition halves so the
       output projection runs K=128 matmuls.
  out = sum_pairs ctxT2_p^T @ Wo_p + bo + residual -> layernorm

V is projected transposed directly (x-chunk stationary, W moving -> psum
[t,512]) so no PE transposes are needed; a strided DVE add scatters each
t-chunk into the per-head fp8 vsum8 blocks with the row bias fused.
"""
import sys
import os

sys.path.insert(0, "/opt/trn_rl_repo")

import numpy as np
import ml_dtypes

import concourse.bass as bass
import concourse.mybir as mybir
import concourse.tile as tile
from concourse import bacc
from concourse import bass_utils

B, S, D = 4, 2048, 512
H, DH = 8, 64
SQ = S // 2
HD = H * DH
EPS = 1e-5
SCALE = 1.0 / np.sqrt(DH)
CSHIFT = 5.5                     # exp shift: attT = exp(s/8 - CSHIFT)
LOG2E = 1.4426950408889634
# Schraudolph constants for e4m3 (3 mantissa bits, bias 7):
#   u8 = round(raw * SCALE*8*log2e + (7*8 - CSHIFT*8*log2e - corr))
SCH_MUL = SCALE * 8.0 * LOG2E
SCH_ADD = 56.0 - CSHIFT * 8.0 * LOG2E - 0.238

F32 = mybir.dt.float32
BF = mybir.dt.bfloat16
F8 = mybir.dt.float8e4
U8 = mybir.dt.uint8
BF_NP = ml_dtypes.bfloat16

_MODULES = {}


def _build_module(reps=1, phases="all"):
    nc = bacc.Bacc("TRN2", target_bir_lowering=False, debug=False)

    # ---- DRAM I/O -----------------------------------------------------------
    d_qdT = nc.dram_tensor("qdT", [D, SQ], BF, kind="ExternalInput")
    d_qtT = nc.dram_tensor("qtT", [D, SQ], BF, kind="ExternalInput")
    d_kdT = nc.dram_tensor("kdT", [D, S], BF, kind="ExternalInput")
    d_ktT = nc.dram_tensor("ktT", [D, S], BF, kind="ExternalInput")
    d_vdT = nc.dram_tensor("vdT", [D, S], BF, kind="ExternalInput")
    d_vtT = nc.dram_tensor("vtT", [D, S], BF, kind="ExternalInput")
    d_qres = nc.dram_tensor("qres", [SQ, D], F32, kind="ExternalInput")
    d_w = {}
    for wn in ("w_qd", "w_qt", "w_kd", "w_kt", "w_vd", "w_vt"):
        d_w[wn] = nc.dram_tensor(wn, [128, 4, D], BF, kind="ExternalInput")
    d_wo = nc.dram_tensor("wo2", [128, 4, D], BF, kind="ExternalInput")
    d_b = {}
    for bn in ("b_qd", "b_qt", "b_kd", "b_kt"):
        d_b[bn] = nc.dram_tensor(bn, [128, 4], F32, kind="ExternalInput")
    d_bv = nc.dram_tensor("b_v", [1, D], F32, kind="ExternalInput")
    d_bo = nc.dram_tensor("bo", [1, D], F32, kind="ExternalInput")
    d_gamma = nc.dram_tensor("gamma", [1, D], F32, kind="ExternalInput")
    d_beta = nc.dram_tensor("beta", [1, D], F32, kind="ExternalInput")
    d_out = nc.dram_tensor("out", [SQ, D], F32, kind="ExternalOutput")

    with tile.TileContext(nc) as tc:
        import contextlib

        with contextlib.ExitStack() as outer:
            resid = outer.enter_context(tc.tile_pool(name="resid", bufs=1))
            kcat = [resid.tile([128, S], BF, tag=f"kcat{h}", name=f"kcat{h}")
                    for h in range(H)]
            qcat = [resid.tile([128, SQ], BF, tag=f"qcat{h}", name=f"qcat{h}")
                    for h in range(H)]
            # vsum8[p, h, tc, col]: col 0:64 = V values (fp8), col 64 = ones,
            # cols 65:128 = zero. DoubleRow lhsT slices are [:, h, 2t:2t+2, :].
            vsum8 = resid.tile([128, H * 16 * 128], F8, tag="vsum8")
            vs4 = vsum8[:, :].rearrange("p (c x) -> p c x", x=128)
            # loop-invariant padding init (outside the rep loop)
            nc.gpsimd.memset(vs4[:, :, 64:128], 0.0)
            nc.gpsimd.memset(vs4[:, :, 64:65], 1.0)

            with contextlib.ExitStack() as top:
                if reps > 1:
                    top.enter_context(tc.For_i(0, reps, 1))
                _emit_body(nc, tc, top, kcat, qcat, vsum8,
                           d_qdT, d_qtT, d_kdT, d_ktT, d_vdT, d_vtT,
                           d_qres, d_w, d_wo, d_b, d_bv, d_bo, d_gamma,
                           d_beta, d_out, phases)

    nc.compile()
    return nc


def _emit_body(nc, tc, top, kcat, qcat, vsum8,
               d_qdT, d_qtT, d_kdT, d_ktT, d_vdT, d_vtT,
               d_qres, d_w, d_wo, d_b, d_bv, d_bo, d_gamma, d_beta, d_out,
               phases="all"):
    import contextlib
    do_proj = phases in ("proj", "projattn", "all")
    do_attn = phases in ("projattn", "all")
    do_out = phases == "all"
    if phases == "none":
        with tc.tile_pool(name="nil", bufs=1) as nil:
            t = nil.tile([128, 512], F32, tag="nil", name="nil")
            nc.sync.dma_start(t[:], d_qres.ap()[0:128, :])
            nc.sync.dma_start(d_out.ap()[0:128, :], t[:])
        return

    Act = mybir.ActivationFunctionType
    Alu = mybir.AluOpType
    Ax = mybir.AxisListType
    PM = mybir.MatmulPerfMode

    consts = top.enter_context(tc.tile_pool(name="consts", bufs=1))

    b_sb = {}
    for bn in ("b_qd", "b_qt", "b_kd", "b_kt"):
        b_sb[bn] = consts.tile([128, 4], F32, tag=f"bias_{bn}", name=f"bias_{bn}")
        nc.sync.dma_start(b_sb[bn][:], d_b[bn].ap())
    bv1 = consts.tile([1, D], F32, tag="bv1")
    nc.sync.dma_start(bv1[:], d_bv.ap())
    bvB = consts.tile([128, D], F32, tag="bvB")
    nc.gpsimd.partition_broadcast(bvB[:], bv1[:])
    # per-partition exp bias (-CSHIFT) for ACT tiles
    expb = consts.tile([128, 1], F32, tag="expb")
    nc.vector.memset(expb[:], -CSHIFT)

    vs4 = vsum8[:, :].rearrange("p (c x) -> p c x", x=128)

    # ---- Phase A: projections ----------------------------------------------
    if not do_proj:
        return
    with (
        tc.tile_pool(name="stage", bufs=1) as stg,
        tc.tile_pool(name="xt", bufs=2) as xtp,
        tc.tile_pool(name="wts", bufs=1) as wtp,
        tc.tile_pool(name="proj_ps", bufs=4, space="PSUM") as pps,
        tc.tile_pool(name="v_ps", bufs=2, space="PSUM") as vps,
    ):
        def load_xt_pair(xT_d, xT_t, sg2, nm):
            xt0 = xtp.tile([128, 4, 1024], BF, tag="xt0", name=f"xt0{nm}")
            xt1 = xtp.tile([128, 4, 1024], BF, tag="xt1", name=f"xt1{nm}")
            nc.sync.dma_start(
                xt0[:], xT_d.ap().rearrange("(kc p) s -> p kc s", p=128)[
                    :, :, sg2 * 1024:(sg2 + 1) * 1024])
            nc.sync.dma_start(
                xt1[:], xT_t.ap().rearrange("(kc p) s -> p kc s", p=128)[
                    :, :, sg2 * 1024:(sg2 + 1) * 1024])
            return xt0, xt1

        # --- V first: vsum8 feeds every head's PV matmuls -------------------
        w_vd_sb = wtp.tile([128, 4, D], BF, tag="wv0", name="wv0")
        w_vt_sb = wtp.tile([128, 4, D], BF, tag="wv1", name="wv1")
        nc.sync.dma_start(w_vd_sb[:], d_w["w_vd"].ap())
        nc.sync.dma_start(w_vt_sb[:], d_w["w_vt"].ap())
        for sg2 in range(2):
            xt0, xt1 = load_xt_pair(d_vdT, d_vtT, sg2, "v")
            for tci in range(8):
                tc_ = sg2 * 8 + tci
                ps = vps.tile([128, 512], F32, tag="vps", name="vps")
                for kc in range(4):
                    nc.tensor.matmul(
                        ps[:], lhsT=xt0[:, kc, tci * 128:(tci + 1) * 128],
                        rhs=w_vd_sb[:, kc, :],
                        start=(kc == 0), stop=False)
                for kc in range(4):
                    nc.tensor.matmul(
                        ps[:], lhsT=xt1[:, kc, tci * 128:(tci + 1) * 128],
                        rhs=w_vt_sb[:, kc, :],
                        start=False, stop=(kc == 3))
                # bias add + fp8 convert + scatter into per-head blocks
                # dst AP: [128, h(8) stride 16*128, 64] over vsum8
                dst = vsum8[:, :].rearrange(
                    "p (h c x) -> p h c x", h=H, c=16)[:, :, tc_, 0:64]
                src = ps[:, :].rearrange("p (h x) -> p h x", h=H)
                nc.vector.tensor_add(dst, src, bvB[:, :].rearrange(
                    "p (h x) -> p h x", h=H))

        # --- K then Q into stream-separated pair-major staging --------------
        KD = stg.tile([128, 4 * S], BF, tag="KD")
        KT = stg.tile([128, 4 * S], BF, tag="KT")
        QD = stg.tile([128, 4 * SQ], BF, tag="QD")
        QT = stg.tile([128, 4 * SQ], BF, tag="QT")

        def cat_proj(xT_d, xT_t, w_d, w_t, bias_d, bias_t, dest_d, dest_t,
                     S_len, nm):
            w_d_sb = wtp.tile([128, 4, D], BF, tag="w0", name=f"w0{nm}")
            w_t_sb = wtp.tile([128, 4, D], BF, tag="w1", name=f"w1{nm}")
            nc.sync.dma_start(w_d_sb[:], w_d.ap())
            nc.sync.dma_start(w_t_sb[:], w_t.ap())
            for sg2 in range(S_len // 1024):
                xt0, xt1 = load_xt_pair(xT_d, xT_t, sg2, nm)
                c0 = sg2 * 1024
                for p in range(4):
                    for (xt, wsb, bsb, dest) in (
                        (xt0, w_d_sb, bias_d, dest_d),
                        (xt1, w_t_sb, bias_t, dest_t),
                    ):
                        ps0 = pps.tile([128, 512], F32, tag="proj", name="ps0")
                        ps1 = pps.tile([128, 512], F32, tag="proj", name="ps1")
                        for kc in range(4):
                            nc.tensor.matmul(
                                ps0[:], lhsT=wsb[:, kc, p * 128:(p + 1) * 128],
                                rhs=xt[:, kc, 0:512],
                                start=(kc == 0), stop=(kc == 3))
                            nc.tensor.matmul(
                                ps1[:], lhsT=wsb[:, kc, p * 128:(p + 1) * 128],
                                rhs=xt[:, kc, 512:1024],
                                start=(kc == 0), stop=(kc == 3))
                        nc.scalar.activation(
                            dest[:, p * S_len + c0:p * S_len + c0 + 512],
                            ps0[:], Act.Identity, bias=bsb[:, p:p + 1])
                        nc.vector.tensor_scalar_add(
                            dest[:, p * S_len + c0 + 512:p * S_len + c0 + 1024],
                            ps1[:], bsb[:, p:p + 1])

        cat_proj(d_kdT, d_ktT, d_w["w_kd"], d_w["w_kt"],
                 b_sb["b_kd"][:], b_sb["b_kt"][:], KD[:], KT[:], S, "k")
        cat_proj(d_qdT, d_qtT, d_w["w_qd"], d_w["w_qt"],
                 b_sb["b_qd"][:], b_sb["b_qt"][:], QD[:], QT[:], SQ, "q")

        # shuffle stream-separated halves into the per-head cat layout:
        # aligned half via DVE (bf16 4x), partition-shifted half via DMA
        for h in range(H):
            hh = h % 2
            p = h // 2
            for (SRC, dpo) in ((KD, 0), (KT, 64)):
                s_ap = SRC[hh * 64:(hh + 1) * 64, p * S:(p + 1) * S]
                d_ap = kcat[h][dpo:dpo + 64, :]
                if hh * 64 == dpo:
                    nc.vector.tensor_copy(d_ap, s_ap)
                else:
                    nc.sync.dma_start(d_ap, s_ap)
            for (SRC, dpo) in ((QD, 0), (QT, 64)):
                s_ap = SRC[hh * 64:(hh + 1) * 64, p * SQ:(p + 1) * SQ]
                d_ap = qcat[h][dpo:dpo + 64, :]
                if hh * 64 == dpo:
                    nc.vector.tensor_copy(d_ap, s_ap)
                else:
                    nc.sync.dma_start(d_ap, s_ap)

    # ---- Phase B: attention + output ---------------------------------------
    if not do_attn:
        return
    with contextlib.ExitStack() as bstk:
        ctxp = bstk.enter_context(tc.tile_pool(name="ctxT", bufs=1))
        wop = bstk.enter_context(tc.tile_pool(name="wo", bufs=1))
        bcp = bstk.enter_context(tc.tile_pool(name="bcast", bufs=1))
        # ctxT2[p, pair, s]: head 2p on partitions 0:64, head 2p+1 on 64:128
        ctxT2 = ctxp.tile([128, 4, SQ], BF, tag="ctxT2")

        wo_sb = wop.tile([128, 4, D], BF, tag="wo")
        nc.sync.dma_start(wo_sb[:], d_wo.ap())
        bo1 = bcp.tile([1, D], F32, tag="bo1")
        ga1 = bcp.tile([1, D], F32, tag="ga1")
        be1 = bcp.tile([1, D], F32, tag="be1")
        nc.sync.dma_start(bo1[:], d_bo.ap())
        nc.sync.dma_start(ga1[:], d_gamma.ap())
        nc.sync.dma_start(be1[:], d_beta.ap())
        boB = bcp.tile([128, D], F32, tag="boB")
        gaB = bcp.tile([128, D], F32, tag="gaB")
        beB = bcp.tile([128, D], F32, tag="beB")
        nc.gpsimd.partition_broadcast(boB[:], bo1[:])
        nc.gpsimd.partition_broadcast(gaB[:], ga1[:])
        nc.gpsimd.partition_broadcast(beB[:], be1[:])
        resb = bcp.tile([128, 8, D], F32, tag="resb")
        for st in range(8):
            qr = bcp.tile([128, D], F32, tag="qr", bufs=2)
            nc.sync.dma_start(qr[:], d_qres.ap()[st * 128:(st + 1) * 128, :])
            nc.gpsimd.tensor_add(resb[:, st, :], qr[:], boB[:])

        with (
            tc.tile_pool(name="at", bufs=5) as atp,
            tc.tile_pool(name="codd", bufs=2) as codp,
            tc.tile_pool(name="rin", bufs=2) as rip,
            tc.tile_pool(name="rb", bufs=2) as rbp,
            tc.tile_pool(name="sc_ps", bufs=3, space="PSUM") as scps,
            tc.tile_pool(name="ctx_ps", bufs=2, space="PSUM") as ctxps,
        ):
            LOOKAHEAD = 2
            exp_ctr = [0]

            def emit_exp(at_ap, sc_ap):
                # split exp between ACT (hw Exp->fp8) and DVE (Schraudolph)
                i = exp_ctr[0]
                exp_ctr[0] += 1
                if i % 16 in (1, 4, 7, 10, 13, 15):
                    nc.vector.tensor_scalar(
                        at_ap.bitcast(U8), sc_ap, SCH_MUL, SCH_ADD,
                        op0=Alu.mult, op1=Alu.add)
                else:
                    nc.scalar.activation(at_ap, sc_ap, Act.Exp,
                                         scale=float(SCALE), bias=expb[:, 0:1])

            def ctx_evac(h, ctx_ps):
                pair = h // 2
                podd = h % 2
                for sk in range(2):
                    rinv = rip.tile([1, 512], F32, tag="rinv", name="rinv")
                    nc.vector.reciprocal(rinv[:], ctx_ps[sk][64:65, :])
                    rb = rbp.tile([64, 512], F32, tag="rb", name="rb")
                    nc.gpsimd.partition_broadcast(rb[:], rinv[:])
                    if podd == 0:
                        nc.vector.tensor_mul(
                            ctxT2[0:64, pair, sk * 512:(sk + 1) * 512],
                            ctx_ps[sk][0:64, :], rb[:])
                    else:
                        co = codp.tile([64, 512], BF, tag="codd", name="codd")
                        nc.vector.tensor_mul(co[:], ctx_ps[sk][0:64, :], rb[:])
                        nc.sync.dma_start(
                            ctxT2[64:128, pair, sk * 512:(sk + 1) * 512], co[:])

            prev_evac = [None]

            for h in range(H):
                ctx_ps = [ctxps.tile([128, 512], F32, tag="ctx", name=f"ctx{sk}")
                          for sk in range(2)]
                ats = [None] * 8
                for step in range(8 + LOOKAHEAD):
                    tcp = step
                    if tcp < 8:
                        at = atp.tile([128, 2, 1024], F8, tag="at", name="at")
                        for i in range(2):
                            tcn = 2 * tcp + i
                            sc = scps.tile([128, 1024], F32, tag="sc", name="sc")
                            for sk in range(2):
                                nc.tensor.matmul(
                                    sc[:, sk * 512:(sk + 1) * 512],
                                    lhsT=kcat[h][:, tcn * 128:(tcn + 1) * 128],
                                    rhs=qcat[h][:, sk * 512:(sk + 1) * 512],
                                    start=True, stop=True)
                            emit_exp(at[:, i, :], sc[:])
                        ats[tcp] = at
                    # deferred evac of the previous head sits between this
                    # head's early exps so DVE isn't head-of-line blocked
                    if step == 1 and prev_evac[0] is not None:
                        ctx_evac(h - 1, prev_evac[0])
                        prev_evac[0] = None
                    pv = step - LOOKAHEAD
                    if pv >= 0:
                        at = ats[pv]
                        for sk in range(2):
                            nc.tensor.matmul(
                                ctx_ps[sk][:],
                                lhsT=vs4[:, 2 * (h * 8 + pv):2 * (h * 8 + pv) + 2, :],
                                rhs=at[:, :, sk * 512:(sk + 1) * 512],
                                start=(pv == 0), stop=(pv == 7),
                                perf_mode=PM.DoubleRow)
                prev_evac[0] = ctx_ps
            ctx_evac(H - 1, prev_evac[0])

        # output projection + residual + layernorm
        if not do_out:
            return
        with (
            tc.tile_pool(name="xs", bufs=2) as xsp,
            tc.tile_pool(name="ss", bufs=2) as ssp,
            tc.tile_pool(name="out_ps", bufs=2, space="PSUM") as ops,
        ):
            for st in range(8):
                po = ops.tile([128, 512], F32, tag="po")
                for pair in range(4):
                    nc.tensor.matmul(
                        po[:],
                        lhsT=ctxT2[:, pair, st * 128:(st + 1) * 128],
                        rhs=wo_sb[:, pair, :], start=(pair == 0), stop=(pair == 3))
                x = xsp.tile([128, D], F32, tag="x")
                nc.vector.tensor_add(x[:], po[:], resb[:, st, :])
                s1 = ssp.tile([128, 1], F32, tag="s1")
                nc.vector.tensor_reduce(s1[:], x[:], axis=Ax.X, op=Alu.add)
                mu = ssp.tile([128, 1], F32, tag="mu")
                nc.vector.tensor_scalar_mul(mu[:], s1[:], 1.0 / D)
                xc = xsp.tile([128, D], F32, tag="xc")
                nc.vector.tensor_scalar_sub(xc[:], x[:], mu[:])
                sq = xsp.tile([128, D], F32, tag="sq")
                ss = ssp.tile([128, 1], F32, tag="ss")
                nc.vector.scalar_tensor_tensor(
                    out=sq[:], in0=xc[:], scalar=1.0, in1=xc[:],
                    op0=Alu.bypass, op1=Alu.mult, accum_out=ss[:])
                var = ssp.tile([128, 1], F32, tag="var")
                nc.vector.tensor_scalar(
                    var[:], ss[:], 1.0 / D, EPS, op0=Alu.mult, op1=Alu.add)
                sd = ssp.tile([128, 1], F32, tag="sd")
                nc.scalar.sqrt(sd[:], var[:])
                rs = ssp.tile([128, 1], F32, tag="rs")
                nc.vector.reciprocal(rs[:], sd[:])
                y = xsp.tile([128, D], F32, tag="y")
                nc.vector.scalar_tensor_tensor(
                    out=y[:], in0=xc[:], scalar=rs[:], in1=gaB[:],
                    op0=Alu.mult, op1=Alu.mult)
                nc.vector.tensor_add(y[:], y[:], beB[:])
                nc.sync.dma_start(d_out.ap()[st * 128:(st + 1) * 128, :], y[:])


def get_module(reps=1):
    import os as _os
    phases = _os.environ.get("KPHASES", "all")
    key = (reps, phases)
    if key not in _MODULES:
        _MODULES[key] = _build_module(reps, phases)
    return _MODULES[key]


def make_in_maps(inputs):
    """Build the 8 per-core input maps from the full problem inputs."""
    w = {}
    for wn, key in (("w_qd", "Wq_d"), ("w_qt", "Wq_t"), ("w_kd", "Wk_d"),
                    ("w_kt", "Wk_t"), ("w_vd", "Wv_d"), ("w_vt", "Wv_t")):
        # [512 in, 512 out] -> [128 p, 4 kc, 512 out]
        w[wn] = np.ascontiguousarray(
            inputs[key].reshape(4, 128, HD).transpose(1, 0, 2)).astype(BF_NP)
    # Wo rows regrouped into head pairs: [128 = (h%2)*64+dh, 4 pairs, D]
    wo2 = np.ascontiguousarray(
        inputs["Wo"].reshape(4, 2, 64, D).transpose(1, 2, 0, 3).reshape(128, 4, D)
    ).astype(BF_NP)

    def bcol(v):
        # [512] -> [128 partition, 4 pair] so column p is the per-partition
        # bias for head-pair p's psum block
        return np.ascontiguousarray(v.reshape(4, 128).T).astype(np.float32)

    b = {
        "b_qd": bcol(inputs["bq_d"]),
        "b_qt": bcol(inputs["bq_t"]),
        "b_kd": bcol(inputs["bk_d"]),
        "b_kt": bcol(inputs["bk_t"]),
    }
    bv = (inputs["bv_d"].astype(np.float32)
          + inputs["bv_t"].astype(np.float32)).reshape(1, D)
    bo = inputs["bo"].reshape(1, D).astype(np.float32)
    gamma = inputs["gamma"].reshape(1, D).astype(np.float32)
    beta = inputs["beta"].reshape(1, D).astype(np.float32)

    kvT = {}
    for name, key in (("kdT", "K_data"), ("ktT", "K_time"),
                      ("vdT", "V_data"), ("vtT", "V_time")):
        kvT[name] = [
            np.ascontiguousarray(
                inputs[key][bb].astype(BF_NP).T) for bb in range(B)]

    in_maps = []
    for c in range(8):
        bb, half = divmod(c, 2)
        sl = slice(half * SQ, (half + 1) * SQ)
        m = {
            "qdT": np.ascontiguousarray(inputs["Q_data"][bb, sl, :].astype(BF_NP).T),
            "qtT": np.ascontiguousarray(inputs["Q_time"][bb, sl, :].astype(BF_NP).T),
            "kdT": kvT["kdT"][bb], "ktT": kvT["ktT"][bb],
            "vdT": kvT["vdT"][bb], "vtT": kvT["vtT"][bb],
            "qres": np.ascontiguousarray(inputs["Q_data"][bb, sl, :].astype(np.float32)),
            "wo2": wo2, "b_v": bv, "bo": bo, "gamma": gamma, "beta": beta,
        }
        m.update(w)
        m.update(b)
        in_maps.append(m)
    return in_maps


def kernel(**inputs):
    inputs = {k: np.asarray(v) for k, v in inputs.items()}
    nc = get_module(reps=1)
    in_maps = make_in_maps(inputs)
    res = bass_utils.run_bass_kernel_spmd(nc, in_maps, core_ids=list(range(8)))
    out = np.empty((B, S, D), dtype=np.float32)
    for c in range(8):
        bb, half = divmod(c, 2)
        out[bb, half * SQ:(half + 1) * SQ, :] = res.results[c]["out"]
    return out


# revision 7
# speedup vs baseline: 1.0816x; 1.0816x over previous
"""Trainium2 Bass kernel for nn_MixAttention (dual-stream attention block).

Sharding: 8 cores = 4 batches x 2 query-halves (data parallel over batch and
sequence). Each core computes K/V projections for its full batch (duplicated
across the 2 cores sharing a batch) and Q projections + attention + output
projection + layernorm for its own 1024 query rows. No collectives needed.

Math per core (Sq=1024 own query rows, Sk=2048 keys of own batch, H=8, DH=64):
  qcat_h = [qd_h; qt_h], kcat_h = [kd_h; kt_h]  (stream-cat on the partition
       axis -> both dual-stream score terms fuse into one K=128 matmul)
  scoresT_h[t,s] = sum_c kcat_h[c,t] qcat_h[c,s]   (computed transposed so
       the PV matmul needs no transposes of the attention matrix)
  attT_h = exp(scoresT_h / 8 - C) in fp8e4m3, C=5.5 keeps the max under the
       e4m3 ceiling; the shift cancels in the softmax normalization. The exp
       is split between the ACT engine (hw Exp) and DVE (Schraudolph bit-cast
       exp: u8 = round(s*log2e + bias), reinterpreted as e4m3).
  ctxU_h = vsum8_h^T @ attT_h via fp8 DoubleRow matmuls (K=256/step, 2x rate);
       a ones column woven into vsum8 col 64 yields softmax sums in psum row
       64 for free.
  ctx_h = ctxU_h * (1/r); head pairs are stacked on partition halves so the
       output projection runs K=128 matmuls.
  out = sum_pairs ctxT2_p^T @ Wo_p + bo + residual -> layernorm

V is projected transposed directly (x-chunk stationary, W moving -> psum
[t,512]) so no PE transposes are needed; a strided DVE add scatters each
t-chunk into the per-head fp8 vsum8 blocks with the row bias fused.
"""
import sys
import os

sys.path.insert(0, "/opt/trn_rl_repo")

import numpy as np
import ml_dtypes

import concourse.bass as bass
import concourse.mybir as mybir
import concourse.tile as tile
from concourse import bacc
from concourse import bass_utils

B, S, D = 4, 2048, 512
H, DH = 8, 64
SQ = S // 2
HD = H * DH
EPS = 1e-5
SCALE = 1.0 / np.sqrt(DH)
CSHIFT = 5.5                     # exp shift: attT = exp(s/8 - CSHIFT)
LOG2E = 1.4426950408889634
# Schraudolph constants for e4m3 (3 mantissa bits, bias 7):
#   u8 = round(raw * SCALE*8*log2e + (7*8 - CSHIFT*8*log2e - corr))
SCH_MUL = SCALE * 8.0 * LOG2E
SCH_ADD = 56.0 - CSHIFT * 8.0 * LOG2E - 0.238

F32 = mybir.dt.float32
BF = mybir.dt.bfloat16
F8 = mybir.dt.float8e4
U8 = mybir.dt.uint8
BF_NP = ml_dtypes.bfloat16

_MODULES = {}


def _build_module(reps=1, phases="all"):
    nc = bacc.Bacc("TRN2", target_bir_lowering=False, debug=False)

    # ---- DRAM I/O -----------------------------------------------------------
    d_qdT = nc.dram_tensor("qdT", [D, SQ], BF, kind="ExternalInput")
    d_qtT = nc.dram_tensor("qtT", [D, SQ], BF, kind="ExternalInput")
    d_kdT = nc.dram_tensor("kdT", [D, S], BF, kind="ExternalInput")
    d_ktT = nc.dram_tensor("ktT", [D, S], BF, kind="ExternalInput")
    d_vdT = nc.dram_tensor("vdT", [D, S], BF, kind="ExternalInput")
    d_vtT = nc.dram_tensor("vtT", [D, S], BF, kind="ExternalInput")
    d_qres = nc.dram_tensor("qres", [SQ, D], F32, kind="ExternalInput")
    d_w = {}
    for wn in ("w_qd", "w_qt", "w_kd", "w_kt", "w_vd", "w_vt"):
        d_w[wn] = nc.dram_tensor(wn, [128, 4, D], BF, kind="ExternalInput")
    d_wo = nc.dram_tensor("wo2", [128, 4, D], BF, kind="ExternalInput")
    d_b = {}
    for bn in ("b_qd", "b_qt", "b_kd", "b_kt"):
        d_b[bn] = nc.dram_tensor(bn, [128, 4], F32, kind="ExternalInput")
    d_bv = nc.dram_tensor("b_v", [1, D], F32, kind="ExternalInput")
    d_bo = nc.dram_tensor("bo", [1, D], F32, kind="ExternalInput")
    d_gamma = nc.dram_tensor("gamma", [1, D], F32, kind="ExternalInput")
    d_beta = nc.dram_tensor("beta", [1, D], F32, kind="ExternalInput")
    d_out = nc.dram_tensor("out", [SQ, D], F32, kind="ExternalOutput")

    with tile.TileContext(nc) as tc:
        import contextlib

        with contextlib.ExitStack() as outer:
            resid = outer.enter_context(tc.tile_pool(name="resid", bufs=1))
            kcat = [resid.tile([128, S], BF, tag=f"kcat{h}", name=f"kcat{h}")
                    for h in range(H)]
            qcat = [resid.tile([128, SQ], BF, tag=f"qcat{h}", name=f"qcat{h}")
                    for h in range(H)]
            # vsum8[p, h, tc, col]: col 0:64 = V values (fp8), col 64 = ones,
            # cols 65:128 = zero. DoubleRow lhsT slices are [:, h, 2t:2t+2, :].
            vsum8 = resid.tile([128, H * 16 * 128], F8, tag="vsum8")
            vs4 = vsum8[:, :].rearrange("p (c x) -> p c x", x=128)
            # loop-invariant padding init (outside the rep loop)
            nc.gpsimd.memset(vs4[:, :, 64:128], 0.0)
            nc.gpsimd.memset(vs4[:, :, 64:65], 1.0)

            with contextlib.ExitStack() as top:
                if reps > 1:
                    top.enter_context(tc.For_i(0, reps, 1))
                _emit_body(nc, tc, top, kcat, qcat, vsum8,
                           d_qdT, d_qtT, d_kdT, d_ktT, d_vdT, d_vtT,
                           d_qres, d_w, d_wo, d_b, d_bv, d_bo, d_gamma,
                           d_beta, d_out, phases)

    nc.compile()
    return nc


def _emit_body(nc, tc, top, kcat, qcat, vsum8,
               d_qdT, d_qtT, d_kdT, d_ktT, d_vdT, d_vtT,
               d_qres, d_w, d_wo, d_b, d_bv, d_bo, d_gamma, d_beta, d_out,
               phases="all"):
    import contextlib
    do_proj = phases in ("proj", "projattn", "all", "projv", "projk")
    do_v = phases != "projk"
    do_kq = phases != "projv"
    do_attn = phases in ("projattn", "all")
    do_out = phases == "all"
    if phases == "none":
        with tc.tile_pool(name="nil", bufs=1) as nil:
            t = nil.tile([128, 512], F32, tag="nil", name="nil")
            nc.sync.dma_start(t[:], d_qres.ap()[0:128, :])
            nc.sync.dma_start(d_out.ap()[0:128, :], t[:])
        return

    Act = mybir.ActivationFunctionType
    Alu = mybir.AluOpType
    Ax = mybir.AxisListType
    PM = mybir.MatmulPerfMode

    consts = top.enter_context(tc.tile_pool(name="consts", bufs=1))

    b_sb = {}
    for bn in ("b_qd", "b_qt", "b_kd", "b_kt"):
        b_sb[bn] = consts.tile([128, 4], F32, tag=f"bias_{bn}", name=f"bias_{bn}")
        nc.sync.dma_start(b_sb[bn][:], d_b[bn].ap())
    bv1 = consts.tile([1, D], F32, tag="bv1")
    nc.sync.dma_start(bv1[:], d_bv.ap())
    bvB = consts.tile([128, D], F32, tag="bvB")
    nc.gpsimd.partition_broadcast(bvB[:], bv1[:])
    # per-partition exp bias (-CSHIFT) for ACT tiles
    expb = consts.tile([128, 1], F32, tag="expb")
    nc.vector.memset(expb[:], -CSHIFT)

    vs4 = vsum8[:, :].rearrange("p (c x) -> p c x", x=128)

    # ---- Phase A: projections ----------------------------------------------
    if not do_proj:
        return
    with (
        tc.tile_pool(name="stage", bufs=1) as stg,
        tc.tile_pool(name="xt", bufs=2) as xtp,
        tc.tile_pool(name="wts", bufs=1) as wtp,
        tc.tile_pool(name="proj_ps", bufs=2, space="PSUM") as pps,
        tc.tile_pool(name="v_ps", bufs=4, space="PSUM") as vps,
    ):
        def load_xt_pair(xT_d, xT_t, sg2, nm):
            xt0 = xtp.tile([128, 4, 1024], BF, tag="xt0", name=f"xt0{nm}")
            xt1 = xtp.tile([128, 4, 1024], BF, tag="xt1", name=f"xt1{nm}")
            nc.sync.dma_start(
                xt0[:], xT_d.ap().rearrange("(kc p) s -> p kc s", p=128)[
                    :, :, sg2 * 1024:(sg2 + 1) * 1024])
            nc.sync.dma_start(
                xt1[:], xT_t.ap().rearrange("(kc p) s -> p kc s", p=128)[
                    :, :, sg2 * 1024:(sg2 + 1) * 1024])
            return xt0, xt1

        # --- V first: vsum8 feeds every head's PV matmuls -------------------
        w_vd_sb = wtp.tile([128, 4, D], BF, tag="wv0", name="wv0")
        w_vt_sb = wtp.tile([128, 4, D], BF, tag="wv1", name="wv1")
        nc.sync.dma_start(w_vd_sb[:], d_w["w_vd"].ap())
        nc.sync.dma_start(w_vt_sb[:], d_w["w_vt"].ap())
        for sg2 in range(2 if do_v else 0):
            xt0, xt1 = load_xt_pair(d_vdT, d_vtT, sg2, "v")
            for tci in range(8):
                tc_ = sg2 * 8 + tci
                ps = vps.tile([128, 512], F32, tag="vps", name="vps")
                for kc in range(4):
                    nc.tensor.matmul(
                        ps[:], lhsT=xt0[:, kc, tci * 128:(tci + 1) * 128],
                        rhs=w_vd_sb[:, kc, :],
                        start=(kc == 0), stop=False)
                for kc in range(4):
                    nc.tensor.matmul(
                        ps[:], lhsT=xt1[:, kc, tci * 128:(tci + 1) * 128],
                        rhs=w_vt_sb[:, kc, :],
                        start=False, stop=(kc == 3))
                # bias add + fp8 convert + scatter into per-head blocks
                # dst AP: [128, h(8) stride 16*128, 64] over vsum8
                dst = vsum8[:, :].rearrange(
                    "p (h c x) -> p h c x", h=H, c=16)[:, :, tc_, 0:64]
                src = ps[:, :].rearrange("p (h x) -> p h x", h=H)
                nc.vector.tensor_add(dst, src, bvB[:, :].rearrange(
                    "p (h x) -> p h x", h=H))

        # --- K then Q into stream-separated pair-major staging --------------
        KD = stg.tile([128, 4 * S], BF, tag="KD")
        KT = stg.tile([128, 4 * S], BF, tag="KT")
        QD = stg.tile([128, 4 * SQ], BF, tag="QD")
        QT = stg.tile([128, 4 * SQ], BF, tag="QT")

        def cat_proj(xT_d, xT_t, w_d, w_t, bias_d, bias_t, dest_d, dest_t,
                     S_len, nm):
            w_d_sb = wtp.tile([128, 4, D], BF, tag="w0", name=f"w0{nm}")
            w_t_sb = wtp.tile([128, 4, D], BF, tag="w1", name=f"w1{nm}")
            nc.sync.dma_start(w_d_sb[:], w_d.ap())
            nc.sync.dma_start(w_t_sb[:], w_t.ap())
            eng = [0]
            for sg2 in range(S_len // 1024):
                xt0, xt1 = load_xt_pair(xT_d, xT_t, sg2, nm)
                c0 = sg2 * 1024
                for p in range(4):
                    for (xt, wsb, bsb, dest) in (
                        (xt0, w_d_sb, bias_d, dest_d),
                        (xt1, w_t_sb, bias_t, dest_t),
                    ):
                        ps = pps.tile([128, 1024], F32, tag="proj", name="ps")
                        for kc in range(4):
                            nc.tensor.matmul(
                                ps[:, 0:512],
                                lhsT=wsb[:, kc, p * 128:(p + 1) * 128],
                                rhs=xt[:, kc, 0:512],
                                start=(kc == 0), stop=(kc == 3))
                            nc.tensor.matmul(
                                ps[:, 512:1024],
                                lhsT=wsb[:, kc, p * 128:(p + 1) * 128],
                                rhs=xt[:, kc, 512:1024],
                                start=(kc == 0), stop=(kc == 3))
                        dst = dest[:, p * S_len + c0:p * S_len + c0 + 1024]
                        if eng[0] % 2 == 0:
                            nc.scalar.activation(
                                dst, ps[:], Act.Identity, bias=bsb[:, p:p + 1])
                        else:
                            nc.vector.tensor_scalar_add(
                                dst, ps[:], bsb[:, p:p + 1])
                        eng[0] += 1

        if do_kq:
            cat_proj(d_kdT, d_ktT, d_w["w_kd"], d_w["w_kt"],
                     b_sb["b_kd"][:], b_sb["b_kt"][:], KD[:], KT[:], S, "k")
            if phases != "projk":
                cat_proj(d_qdT, d_qtT, d_w["w_qd"], d_w["w_qt"],
                         b_sb["b_qd"][:], b_sb["b_qt"][:], QD[:], QT[:], SQ, "q")

        # shuffle stream-separated halves into the per-head cat layout:
        # aligned half via DVE (bf16 4x), partition-shifted half via DMA
        for h in range(H if do_kq and phases != "projk" else 0):
            hh = h % 2
            p = h // 2
            for (SRC, dpo) in ((KD, 0), (KT, 64)):
                s_ap = SRC[hh * 64:(hh + 1) * 64, p * S:(p + 1) * S]
                d_ap = kcat[h][dpo:dpo + 64, :]
                if hh * 64 == dpo:
                    nc.vector.tensor_copy(d_ap, s_ap)
                else:
                    nc.sync.dma_start(d_ap, s_ap)
            for (SRC, dpo) in ((QD, 0), (QT, 64)):
                s_ap = SRC[hh * 64:(hh + 1) * 64, p * SQ:(p + 1) * SQ]
                d_ap = qcat[h][dpo:dpo + 64, :]
                if hh * 64 == dpo:
                    nc.vector.tensor_copy(d_ap, s_ap)
                else:
                    nc.sync.dma_start(d_ap, s_ap)

    # ---- Phase B: attention + output ---------------------------------------
    if not do_attn:
        return
    with contextlib.ExitStack() as bstk:
        ctxp = bstk.enter_context(tc.tile_pool(name="ctxT", bufs=1))
        wop = bstk.enter_context(tc.tile_pool(name="wo", bufs=1))
        bcp = bstk.enter_context(tc.tile_pool(name="bcast", bufs=1))
        # ctxT2[p, pair, s]: head 2p on partitions 0:64, head 2p+1 on 64:128
        ctxT2 = ctxp.tile([128, 4, SQ], BF, tag="ctxT2")

        wo_sb = wop.tile([128, 4, D], BF, tag="wo")
        nc.sync.dma_start(wo_sb[:], d_wo.ap())
        bo1 = bcp.tile([1, D], F32, tag="bo1")
        ga1 = bcp.tile([1, D], F32, tag="ga1")
        be1 = bcp.tile([1, D], F32, tag="be1")
        nc.sync.dma_start(bo1[:], d_bo.ap())
        nc.sync.dma_start(ga1[:], d_gamma.ap())
        nc.sync.dma_start(be1[:], d_beta.ap())
        boB = bcp.tile([128, D], F32, tag="boB")
        gaB = bcp.tile([128, D], F32, tag="gaB")
        beB = bcp.tile([128, D], F32, tag="beB")
        nc.gpsimd.partition_broadcast(boB[:], bo1[:])
        nc.gpsimd.partition_broadcast(gaB[:], ga1[:])
        nc.gpsimd.partition_broadcast(beB[:], be1[:])
        resb = bcp.tile([128, 8, D], F32, tag="resb")
        for st in range(8):
            qr = bcp.tile([128, D], F32, tag="qr", bufs=2)
            nc.sync.dma_start(qr[:], d_qres.ap()[st * 128:(st + 1) * 128, :])
            nc.gpsimd.tensor_add(resb[:, st, :], qr[:], boB[:])

        with (
            tc.tile_pool(name="at", bufs=5) as atp,
            tc.tile_pool(name="codd", bufs=2) as codp,
            tc.tile_pool(name="rin", bufs=2) as rip,
            tc.tile_pool(name="rb", bufs=2) as rbp,
            tc.tile_pool(name="sc_ps", bufs=3, space="PSUM") as scps,
            tc.tile_pool(name="ctx_ps", bufs=2, space="PSUM") as ctxps,
        ):
            LOOKAHEAD = 3
            exp_ctr = [0]

            def emit_exp(at_ap, sc_ap):
                # split exp between ACT (hw Exp->fp8) and DVE (Schraudolph)
                i = exp_ctr[0]
                exp_ctr[0] += 1
                if i % 16 in (1, 3, 6, 8, 11, 13, 15):
                    nc.vector.tensor_scalar(
                        at_ap.bitcast(U8), sc_ap, SCH_MUL, SCH_ADD,
                        op0=Alu.mult, op1=Alu.add)
                else:
                    nc.scalar.activation(at_ap, sc_ap, Act.Exp,
                                         scale=float(SCALE), bias=expb[:, 0:1])

            def ctx_evac(h, ctx_ps):
                pair = h // 2
                podd = h % 2
                for sk in range(2):
                    rinv = rip.tile([1, 512], F32, tag="rinv", name="rinv")
                    nc.vector.reciprocal(rinv[:], ctx_ps[sk][64:65, :])
                    rb = rbp.tile([64, 512], F32, tag="rb", name="rb")
                    nc.gpsimd.partition_broadcast(rb[:], rinv[:])
                    if podd == 0:
                        nc.vector.tensor_mul(
                            ctxT2[0:64, pair, sk * 512:(sk + 1) * 512],
                            ctx_ps[sk][0:64, :], rb[:])
                    else:
                        co = codp.tile([64, 512], BF, tag="codd", name="codd")
                        nc.vector.tensor_mul(co[:], ctx_ps[sk][0:64, :], rb[:])
                        nc.sync.dma_start(
                            ctxT2[64:128, pair, sk * 512:(sk + 1) * 512], co[:])

            prev_evac = [None]

            for h in range(H):
                ctx_ps = [ctxps.tile([128, 512], F32, tag="ctx", name=f"ctx{sk}")
                          for sk in range(2)]
                ats = [None] * 8
                for step in range(8 + LOOKAHEAD):
                    tcp = step
                    if tcp < 8:
                        at = atp.tile([128, 2, 1024], F8, tag="at", name="at")
                        for i in range(2):
                            tcn = 2 * tcp + i
                            sc = scps.tile([128, 1024], F32, tag="sc", name="sc")
                            for sk in range(2):
                                nc.tensor.matmul(
                                    sc[:, sk * 512:(sk + 1) * 512],
                                    lhsT=kcat[h][:, tcn * 128:(tcn + 1) * 128],
                                    rhs=qcat[h][:, sk * 512:(sk + 1) * 512],
                                    start=True, stop=True)
                            emit_exp(at[:, i, :], sc[:])
                        ats[tcp] = at
                    # deferred evac of the previous head sits between this
                    # head's early exps so DVE isn't head-of-line blocked
                    if step == 1 and prev_evac[0] is not None:
                        ctx_evac(h - 1, prev_evac[0])
                        prev_evac[0] = None
                    pv = step - LOOKAHEAD
                    if pv >= 0:
                        at = ats[pv]
                        for sk in range(2):
                            nc.tensor.matmul(
                                ctx_ps[sk][:],
                                lhsT=vs4[:, 2 * (h * 8 + pv):2 * (h * 8 + pv) + 2, :],
                                rhs=at[:, :, sk * 512:(sk + 1) * 512],
                                start=(pv == 0), stop=(pv == 7),
                                perf_mode=PM.DoubleRow)
                prev_evac[0] = ctx_ps
            ctx_evac(H - 1, prev_evac[0])

        # output projection + residual + layernorm
        if not do_out:
            return
        with (
            tc.tile_pool(name="xs", bufs=2) as xsp,
            tc.tile_pool(name="ss", bufs=2) as ssp,
            tc.tile_pool(name="out_ps", bufs=4, space="PSUM") as ops,
        ):
            for st in range(8):
                po = ops.tile([128, 512], F32, tag="po")
                for pair in range(4):
                    nc.tensor.matmul(
                        po[:],
                        lhsT=ctxT2[:, pair, st * 128:(st + 1) * 128],
                        rhs=wo_sb[:, pair, :], start=(pair == 0), stop=(pair == 3))
                x = xsp.tile([128, D], F32, tag="x")
                nc.vector.tensor_add(x[:], po[:], resb[:, st, :])
                s1 = ssp.tile([128, 1], F32, tag="s1")
                nc.vector.tensor_reduce(s1[:], x[:], axis=Ax.X, op=Alu.add)
                mu = ssp.tile([128, 1], F32, tag="mu")
                nc.vector.tensor_scalar_mul(mu[:], s1[:], 1.0 / D)
                xc = xsp.tile([128, D], F32, tag="xc")
                nc.vector.tensor_scalar_sub(xc[:], x[:], mu[:])
                sq = xsp.tile([128, D], F32, tag="sq")
                ss = ssp.tile([128, 1], F32, tag="ss")
                nc.vector.scalar_tensor_tensor(
                    out=sq[:], in0=xc[:], scalar=1.0, in1=xc[:],
                    op0=Alu.bypass, op1=Alu.mult, accum_out=ss[:])
                var = ssp.tile([128, 1], F32, tag="var")
                nc.vector.tensor_scalar(
                    var[:], ss[:], 1.0 / D, EPS, op0=Alu.mult, op1=Alu.add)
                sd = ssp.tile([128, 1], F32, tag="sd")
                nc.scalar.sqrt(sd[:], var[:])
                rs = ssp.tile([128, 1], F32, tag="rs")
                nc.vector.reciprocal(rs[:], sd[:])
                y = xsp.tile([128, D], F32, tag="y")
                nc.vector.scalar_tensor_tensor(
                    out=y[:], in0=xc[:], scalar=rs[:], in1=gaB[:],
                    op0=Alu.mult, op1=Alu.mult)
                nc.vector.tensor_add(y[:], y[:], beB[:])
                nc.sync.dma_start(d_out.ap()[st * 128:(st + 1) * 128, :], y[:])


def get_module(reps=1):
    import os as _os
    phases = _os.environ.get("KPHASES", "all")
    key = (reps, phases)
    if key not in _MODULES:
        _MODULES[key] = _build_module(reps, phases)
    return _MODULES[key]


def make_in_maps(inputs):
    """Build the 8 per-core input maps from the full problem inputs."""
    w = {}
    for wn, key in (("w_qd", "Wq_d"), ("w_qt", "Wq_t"), ("w_kd", "Wk_d"),
                    ("w_kt", "Wk_t"), ("w_vd", "Wv_d"), ("w_vt", "Wv_t")):
        # [512 in, 512 out] -> [128 p, 4 kc, 512 out]
        w[wn] = np.ascontiguousarray(
            inputs[key].reshape(4, 128, HD).transpose(1, 0, 2)).astype(BF_NP)
    # Wo rows regrouped into head pairs: [128 = (h%2)*64+dh, 4 pairs, D]
    wo2 = np.ascontiguousarray(
        inputs["Wo"].reshape(4, 2, 64, D).transpose(1, 2, 0, 3).reshape(128, 4, D)
    ).astype(BF_NP)

    def bcol(v):
        # [512] -> [128 partition, 4 pair] so column p is the per-partition
        # bias for head-pair p's psum block
        return np.ascontiguousarray(v.reshape(4, 128).T).astype(np.float32)

    b = {
        "b_qd": bcol(inputs["bq_d"]),
        "b_qt": bcol(inputs["bq_t"]),
        "b_kd": bcol(inputs["bk_d"]),
        "b_kt": bcol(inputs["bk_t"]),
    }
    bv = (inputs["bv_d"].astype(np.float32)
          + inputs["bv_t"].astype(np.float32)).reshape(1, D)
    bo = inputs["bo"].reshape(1, D).astype(np.float32)
    gamma = inputs["gamma"].reshape(1, D).astype(np.float32)
    beta = inputs["beta"].reshape(1, D).astype(np.float32)

    kvT = {}
    for name, key in (("kdT", "K_data"), ("ktT", "K_time"),
                      ("vdT", "V_data"), ("vtT", "V_time")):
        kvT[name] = [
            np.ascontiguousarray(
                inputs[key][bb].astype(BF_NP).T) for bb in range(B)]

    in_maps = []
    for c in range(8):
        bb, half = divmod(c, 2)
        sl = slice(half * SQ, (half + 1) * SQ)
        m = {
            "qdT": np.ascontiguousarray(inputs["Q_data"][bb, sl, :].astype(BF_NP).T),
            "qtT": np.ascontiguousarray(inputs["Q_time"][bb, sl, :].astype(BF_NP).T),
            "kdT": kvT["kdT"][bb], "ktT": kvT["ktT"][bb],
            "vdT": kvT["vdT"][bb], "vtT": kvT["vtT"][bb],
            "qres": np.ascontiguousarray(inputs["Q_data"][bb, sl, :].astype(np.float32)),
            "wo2": wo2, "b_v": bv, "bo": bo, "gamma": gamma, "beta": beta,
        }
        m.update(w)
        m.update(b)
        in_maps.append(m)
    return in_maps


def kernel(**inputs):
    inputs = {k: np.asarray(v) for k, v in inputs.items()}
    nc = get_module(reps=1)
    in_maps = make_in_maps(inputs)
    res = bass_utils.run_bass_kernel_spmd(nc, in_maps, core_ids=list(range(8)))
    out = np.empty((B, S, D), dtype=np.float32)
    for c in range(8):
        bb, half = divmod(c, 2)
        out[bb, half * SQ:(half + 1) * SQ, :] = res.results[c]["out"]
    return out


# revision 17
# speedup vs baseline: 1.1296x; 1.0444x over previous
"""Trainium2 Bass kernel for nn_MixAttention (dual-stream attention block).

Sharding: 8 cores = 4 batches x 2 query-halves (data parallel over batch and
sequence). Each core computes K/V projections for its full batch (duplicated
across the 2 cores sharing a batch) and Q projections + attention + output
projection + layernorm for its own 1024 query rows. No collectives needed.

Math per core (Sq=1024 own query rows, Sk=2048 keys of own batch, H=8, DH=64):
  qcat_h = [qd_h; qt_h], kcat_h = [kd_h; kt_h]  (stream-cat on the partition
       axis -> both dual-stream score terms fuse into one K=128 matmul)
  scoresT_h[t,s] = sum_c kcat_h[c,t] qcat_h[c,s]   (computed transposed so
       the PV matmul needs no transposes of the attention matrix)
  attT_h = exp(scoresT_h / 8 - C) in fp8e4m3, C=5.5 keeps the max under the
       e4m3 ceiling; the shift cancels in the softmax normalization. The exp
       is split between the ACT engine (hw Exp) and DVE (Schraudolph bit-cast
       exp: u8 = round(s*log2e + bias), reinterpreted as e4m3).
  ctxU_h = vsum8_h^T @ attT_h via fp8 DoubleRow matmuls (K=256/step, 2x rate);
       a ones column woven into vsum8 col 64 yields softmax sums in psum row
       64 for free.
  ctx_h = ctxU_h * (1/r); head pairs are stacked on partition halves so the
       output projection runs K=128 matmuls.
  out = sum_pairs ctxT2_p^T @ Wo_p + bo + residual -> layernorm

V is projected transposed directly (x-chunk stationary, W moving -> psum
[t,512]) so no PE transposes are needed; a strided DVE add scatters each
t-chunk into the per-head fp8 vsum8 blocks with the row bias fused.
"""
import sys
import os

sys.path.insert(0, "/opt/trn_rl_repo")

import numpy as np
import ml_dtypes

import concourse.bass as bass
import concourse.mybir as mybir
import concourse.tile as tile
from concourse import bacc
from concourse import bass_utils

B, S, D = 4, 2048, 512
H, DH = 8, 64
SQ = S // 2
HD = H * DH
EPS = 1e-5
SCALE = 1.0 / np.sqrt(DH)
CSHIFT = 5.5                     # exp shift: attT = exp(s/8 - CSHIFT)
LOG2E = 1.4426950408889634
# Schraudolph constants for e4m3 (3 mantissa bits, bias 7):
#   u8 = round(raw * SCALE*8*log2e + (7*8 - CSHIFT*8*log2e - corr))
SCH_MUL = SCALE * 8.0 * LOG2E
SCH_ADD = 56.0 - CSHIFT * 8.0 * LOG2E - 0.238

F32 = mybir.dt.float32
BF = mybir.dt.bfloat16
F8 = mybir.dt.float8e4
U8 = mybir.dt.uint8
BF_NP = ml_dtypes.bfloat16

_MODULES = {}


def _build_module(reps=1, phases="all"):
    nc = bacc.Bacc("TRN2", target_bir_lowering=False, debug=False)

    # ---- DRAM I/O -----------------------------------------------------------
    d_qdT = nc.dram_tensor("qdT", [D, SQ], BF, kind="ExternalInput")
    d_qtT = nc.dram_tensor("qtT", [D, SQ], BF, kind="ExternalInput")
    d_kdT = nc.dram_tensor("kdT", [D, S], BF, kind="ExternalInput")
    d_ktT = nc.dram_tensor("ktT", [D, S], BF, kind="ExternalInput")
    d_vdT = nc.dram_tensor("vdT", [D, S], BF, kind="ExternalInput")
    d_vtT = nc.dram_tensor("vtT", [D, S], BF, kind="ExternalInput")
    d_qres = nc.dram_tensor("qres", [SQ, D], F32, kind="ExternalInput")
    d_w = {}
    for wn in ("w_qd", "w_qt", "w_kd", "w_kt", "w_vd", "w_vt"):
        d_w[wn] = nc.dram_tensor(wn, [128, 4, D], BF, kind="ExternalInput")
    d_wo = nc.dram_tensor("wo2", [128, 4, D], BF, kind="ExternalInput")
    d_b = {}
    for bn in ("b_qd", "b_qt", "b_kd", "b_kt"):
        d_b[bn] = nc.dram_tensor(bn, [128, 4], F32, kind="ExternalInput")
    d_bv = nc.dram_tensor("b_v", [1, D], F32, kind="ExternalInput")
    d_bo = nc.dram_tensor("bo", [1, D], F32, kind="ExternalInput")
    d_gamma = nc.dram_tensor("gamma", [1, D], F32, kind="ExternalInput")
    d_beta = nc.dram_tensor("beta", [1, D], F32, kind="ExternalInput")
    d_out = nc.dram_tensor("out", [SQ, D], F32, kind="ExternalOutput")

    with tile.TileContext(nc) as tc:
        import contextlib

        with contextlib.ExitStack() as outer:
            resid = outer.enter_context(tc.tile_pool(name="resid", bufs=1))
            kcat = [resid.tile([128, S], BF, tag=f"kcat{h}", name=f"kcat{h}")
                    for h in range(H)]
            qcat = [resid.tile([128, SQ], BF, tag=f"qcat{h}", name=f"qcat{h}")
                    for h in range(H)]
            # vsum8[p, h, tc, col]: col 0:64 = V values (fp8), col 64 = ones,
            # cols 65:128 = zero. DoubleRow lhsT slices are [:, h, 2t:2t+2, :].
            vsum8 = resid.tile([128, H * 16 * 128], F8, tag="vsum8")
            vs4 = vsum8[:, :].rearrange("p (c x) -> p c x", x=128)
            # loop-invariant padding init (outside the rep loop)
            nc.gpsimd.memset(vs4[:, :, 64:128], 0.0)
            nc.gpsimd.memset(vs4[:, :, 64:65], 1.0)
            if phases.startswith("attn"):
                for h in range(H):
                    nc.gpsimd.memset(kcat[h][:], 0.1)
                    nc.gpsimd.memset(qcat[h][:], 0.1)
                nc.gpsimd.memset(vs4[:, :, 0:64], 0.5)

            with contextlib.ExitStack() as top:
                if reps > 1:
                    top.enter_context(tc.For_i(0, reps, 1))
                _emit_body(nc, tc, top, kcat, qcat, vsum8,
                           d_qdT, d_qtT, d_kdT, d_ktT, d_vdT, d_vtT,
                           d_qres, d_w, d_wo, d_b, d_bv, d_bo, d_gamma,
                           d_beta, d_out, phases)

    nc.compile()
    return nc


def _emit_body(nc, tc, top, kcat, qcat, vsum8,
               d_qdT, d_qtT, d_kdT, d_ktT, d_vdT, d_vtT,
               d_qres, d_w, d_wo, d_b, d_bv, d_bo, d_gamma, d_beta, d_out,
               phases="all"):
    import contextlib
    do_proj = phases in ("proj", "projattn", "all", "projv", "projk")
    if phases.startswith("attn"):
        do_proj = False
    do_v = phases != "projk"
    do_kq = phases != "projv"
    do_attn = phases in ("projattn", "all", "attnonly") or phases.startswith("attn")
    attn_exp = not phases.endswith("sc")
    attn_pv = phases.startswith("attnpv") or phases in ("projattn", "all", "attnonly", "attnnoout")
    attn_evac = phases != "attnpve"
    evac_nb = phases == "attnpvnb"      # skip Pool broadcast
    evac_nd = phases == "attnpvnd"      # skip odd-head DMA
    exp_force = ("act" if phases.endswith("act") else
                 "dve" if phases.endswith("dve") else None)
    do_out = phases in ("all", "attnonly")
    # attnnoout: attention + evac without out-proj/LN
    if phases == "none":
        with tc.tile_pool(name="nil", bufs=1) as nil:
            t = nil.tile([128, 512], F32, tag="nil", name="nil")
            nc.sync.dma_start(t[:], d_qres.ap()[0:128, :])
            nc.sync.dma_start(d_out.ap()[0:128, :], t[:])
        return

    Act = mybir.ActivationFunctionType
    Alu = mybir.AluOpType
    Ax = mybir.AxisListType
    PM = mybir.MatmulPerfMode

    consts = top.enter_context(tc.tile_pool(name="consts", bufs=1))

    b_sb = {}
    for bn in ("b_qd", "b_qt", "b_kd", "b_kt"):
        b_sb[bn] = consts.tile([128, 4], F32, tag=f"bias_{bn}", name=f"bias_{bn}")
        nc.sync.dma_start(b_sb[bn][:], d_b[bn].ap())
    bv1 = consts.tile([1, D], F32, tag="bv1")
    nc.sync.dma_start(bv1[:], d_bv.ap())
    bvB = consts.tile([128, D], F32, tag="bvB")
    nc.gpsimd.partition_broadcast(bvB[:], bv1[:])
    # per-partition exp bias (-CSHIFT) for ACT tiles
    expb = consts.tile([128, 1], F32, tag="expb")
    nc.vector.memset(expb[:], -CSHIFT)

    vs4 = vsum8[:, :].rearrange("p (c x) -> p c x", x=128)

    # ---- Phase A: projections ----------------------------------------------
    with (
        tc.tile_pool(name="stage", bufs=1) as stg,
        tc.tile_pool(name="xt", bufs=2) as xtp,
        tc.tile_pool(name="wts", bufs=1) as wtp,
        tc.tile_pool(name="proj_ps", bufs=2, space="PSUM") as pps,
        tc.tile_pool(name="v_ps", bufs=4, space="PSUM") as vps,
    ):
        def load_xt_pair(xT_d, xT_t, sg2, nm):
            xt0 = xtp.tile([128, 4, 1024], BF, tag="xt0", name=f"xt0{nm}")
            xt1 = xtp.tile([128, 4, 1024], BF, tag="xt1", name=f"xt1{nm}")
            nc.sync.dma_start(
                xt0[:], xT_d.ap().rearrange("(kc p) s -> p kc s", p=128)[
                    :, :, sg2 * 1024:(sg2 + 1) * 1024])
            nc.sync.dma_start(
                xt1[:], xT_t.ap().rearrange("(kc p) s -> p kc s", p=128)[
                    :, :, sg2 * 1024:(sg2 + 1) * 1024])
            return xt0, xt1

        # --- V first: vsum8 feeds every head's PV matmuls -------------------
        if do_proj and do_v:
            w_vd_sb = wtp.tile([128, 4, D], BF, tag="wv0", name="wv0")
            w_vt_sb = wtp.tile([128, 4, D], BF, tag="wv1", name="wv1")
            nc.sync.dma_start(w_vd_sb[:], d_w["w_vd"].ap())
            nc.sync.dma_start(w_vt_sb[:], d_w["w_vt"].ap())
        for sg2 in range(2 if (do_v and do_proj) else 0):
            xt0, xt1 = load_xt_pair(d_vdT, d_vtT, sg2, "v")
            for tci in range(8):
                tc_ = sg2 * 8 + tci
                ps = vps.tile([128, 512], F32, tag="vps", name="vps")
                for kc in range(4):
                    nc.tensor.matmul(
                        ps[:], lhsT=xt0[:, kc, tci * 128:(tci + 1) * 128],
                        rhs=w_vd_sb[:, kc, :],
                        start=(kc == 0), stop=False)
                for kc in range(4):
                    nc.tensor.matmul(
                        ps[:], lhsT=xt1[:, kc, tci * 128:(tci + 1) * 128],
                        rhs=w_vt_sb[:, kc, :],
                        start=False, stop=(kc == 3))
                # bias add + fp8 convert + scatter into per-head blocks
                # dst AP: [128, h(8) stride 16*128, 64] over vsum8
                dst = vsum8[:, :].rearrange(
                    "p (h c x) -> p h c x", h=H, c=16)[:, :, tc_, 0:64]
                src = ps[:, :].rearrange("p (h x) -> p h x", h=H)
                nc.vector.tensor_add(dst, src, bvB[:, :].rearrange(
                    "p (h x) -> p h x", h=H))

        # --- K then Q into stream-separated pair-major staging --------------
        KD = stg.tile([128, 4 * S], BF, tag="KD")
        KT = stg.tile([128, 4 * S], BF, tag="KT")
        QD = stg.tile([128, 4 * SQ], BF, tag="QD")
        QT = stg.tile([128, 4 * SQ], BF, tag="QT")

        def cat_proj(xT_d, xT_t, w_d, w_t, bias_d, bias_t, dest_d, dest_t,
                     S_len, nm):
            w_d_sb = wtp.tile([128, 4, D], BF, tag="w0", name=f"w0{nm}")
            w_t_sb = wtp.tile([128, 4, D], BF, tag="w1", name=f"w1{nm}")
            nc.sync.dma_start(w_d_sb[:], w_d.ap())
            nc.sync.dma_start(w_t_sb[:], w_t.ap())
            eng = [0]
            for sg2 in range(S_len // 1024):
                xt0, xt1 = load_xt_pair(xT_d, xT_t, sg2, nm)
                c0 = sg2 * 1024
                for p in range(4):
                    for (xt, wsb, bsb, dest) in (
                        (xt0, w_d_sb, bias_d, dest_d),
                        (xt1, w_t_sb, bias_t, dest_t),
                    ):
                        ps = pps.tile([128, 1024], F32, tag="proj", name="ps")
                        for kc in range(4):
                            nc.tensor.matmul(
                                ps[:, 0:512],
                                lhsT=wsb[:, kc, p * 128:(p + 1) * 128],
                                rhs=xt[:, kc, 0:512],
                                start=(kc == 0), stop=(kc == 3))
                            nc.tensor.matmul(
                                ps[:, 512:1024],
                                lhsT=wsb[:, kc, p * 128:(p + 1) * 128],
                                rhs=xt[:, kc, 512:1024],
                                start=(kc == 0), stop=(kc == 3))
                        dst = dest[:, p * S_len + c0:p * S_len + c0 + 1024]
                        if eng[0] % 2 == 0:
                            nc.scalar.activation(
                                dst, ps[:], Act.Identity, bias=bsb[:, p:p + 1])
                        else:
                            nc.vector.tensor_scalar_add(
                                dst, ps[:], bsb[:, p:p + 1])
                        eng[0] += 1

        if do_kq and do_proj:
            cat_proj(d_kdT, d_ktT, d_w["w_kd"], d_w["w_kt"],
                     b_sb["b_kd"][:], b_sb["b_kt"][:], KD[:], KT[:], S, "k")
            if phases != "projk":
                cat_proj(d_qdT, d_qtT, d_w["w_qd"], d_w["w_qt"],
                         b_sb["b_qd"][:], b_sb["b_qt"][:], QD[:], QT[:], SQ, "q")

        # shuffle stream-separated halves into the per-head cat layout:
        # aligned half via DVE (bf16 4x), partition-shifted half via DMA
        for h in range(H if (do_proj and do_kq and phases != "projk") else 0):
            hh = h % 2
            p = h // 2
            for (SRC, dpo) in ((KD, 0), (KT, 64)):
                s_ap = SRC[hh * 64:(hh + 1) * 64, p * S:(p + 1) * S]
                d_ap = kcat[h][dpo:dpo + 64, :]
                if hh * 64 == dpo:
                    nc.vector.tensor_copy(d_ap, s_ap)
                else:
                    nc.sync.dma_start(d_ap, s_ap)
            for (SRC, dpo) in ((QD, 0), (QT, 64)):
                s_ap = SRC[hh * 64:(hh + 1) * 64, p * SQ:(p + 1) * SQ]
                d_ap = qcat[h][dpo:dpo + 64, :]
                if hh * 64 == dpo:
                    nc.vector.tensor_copy(d_ap, s_ap)
                else:
                    nc.sync.dma_start(d_ap, s_ap)

    # ---- Phase B: attention + output ---------------------------------------
    if not do_attn:
        return
    with contextlib.ExitStack() as bstk:
        ctxp = bstk.enter_context(tc.tile_pool(name="ctxT", bufs=1))
        wop = bstk.enter_context(tc.tile_pool(name="wo", bufs=1))
        bcp = bstk.enter_context(tc.tile_pool(name="bcast", bufs=1))
        # ctxT2[p, pair, s]: head 2p on partitions 0:64, head 2p+1 on 64:128
        ctxT2 = ctxp.tile([128, 4, SQ], BF, tag="ctxT2")

        wo_sb = wop.tile([128, 4, D], BF, tag="wo")
        nc.sync.dma_start(wo_sb[:], d_wo.ap())
        bo1 = bcp.tile([1, D], F32, tag="bo1")
        ga1 = bcp.tile([1, D], F32, tag="ga1")
        be1 = bcp.tile([1, D], F32, tag="be1")
        nc.sync.dma_start(bo1[:], d_bo.ap())
        nc.sync.dma_start(ga1[:], d_gamma.ap())
        nc.sync.dma_start(be1[:], d_beta.ap())
        boB = bcp.tile([128, D], F32, tag="boB")
        gaB = bcp.tile([128, D], F32, tag="gaB")
        beB = bcp.tile([128, D], F32, tag="beB")
        nc.gpsimd.partition_broadcast(boB[:], bo1[:])
        nc.gpsimd.partition_broadcast(gaB[:], ga1[:])
        nc.gpsimd.partition_broadcast(beB[:], be1[:])
        resb = bcp.tile([128, 8, D], F32, tag="resb")
        for st in range(8):
            qr = bcp.tile([128, D], F32, tag="qr", bufs=2)
            nc.sync.dma_start(qr[:], d_qres.ap()[st * 128:(st + 1) * 128, :])
            nc.gpsimd.tensor_add(resb[:, st, :], qr[:], boB[:])

        with (
            tc.tile_pool(name="at", bufs=5) as atp,
            tc.tile_pool(name="codd", bufs=1) as codp,
            tc.tile_pool(name="rin", bufs=2) as rip,
            tc.tile_pool(name="rb", bufs=2) as rbp,
            tc.tile_pool(name="sc_ps", bufs=2, space="PSUM") as scps,
            tc.tile_pool(name="ctx_ps", bufs=4, space="PSUM") as ctxps,
        ):
            LOOKAHEAD = 3
            exp_ctr = [0]

            def emit_exp(at_ap, sc_ap):
                # split exp between ACT (hw Exp->fp8) and DVE (Schraudolph)
                i = exp_ctr[0]
                exp_ctr[0] += 1
                if not attn_exp:
                    return
                dve = (i % 16 in (1, 4, 7, 10, 13, 15))
                if exp_force == "act":
                    dve = False
                elif exp_force == "dve":
                    dve = True
                if dve:
                    nc.vector.tensor_scalar(
                        at_ap.bitcast(U8), sc_ap, SCH_MUL, SCH_ADD,
                        op0=Alu.mult, op1=Alu.add)
                else:
                    nc.scalar.activation(at_ap, sc_ap, Act.Exp,
                                         scale=float(SCALE), bias=expb[:, 0:1])

            def evac_stage1(h, ctx_ps):
                rbs = []
                for sk in range(2):
                    rrow = rip.tile([1, 512], F32, tag="rrow", name="rrow")
                    nc.vector.tensor_copy(rrow[:], ctx_ps[sk][64:65, :])
                    rinv = rip.tile([1, 512], F32, tag="rinv", name="rinv")
                    nc.vector.reciprocal_approx_fast(rinv[:], rrow[:])
                    if evac_nb:
                        rbs.append(bvB[0:64, :])
                    else:
                        rbt = rbp.tile([64, 512], F32, tag="rb", name="rb")
                        nc.gpsimd.partition_broadcast(rbt[:], rinv[:])
                        rbs.append(rbt[:])
                return rbs

            def evac_stage2(h, ctx_ps, rbs):
                pair = h // 2
                podd = h % 2
                for sk in range(2):
                    if podd == 0 or evac_nd:
                        nc.vector.tensor_mul(
                            ctxT2[0:64, pair, sk * 512:(sk + 1) * 512],
                            ctx_ps[sk][0:64, :], rbs[sk])
                    else:
                        nc.vector.tensor_mul(
                            co4[0:64, pair, sk * 512:(sk + 1) * 512],
                            ctx_ps[sk][0:64, :], rbs[sk])

            def ctx_evac(h, ctx_ps):
                evac_stage2(h, ctx_ps, evac_stage1(h, ctx_ps))

            prev_evac = [None]
            prev_rbs = [None]
            co4 = codp.tile([64, 4, SQ], BF, tag="co4")

            for h in range(H):
                ctx_ps = [ctxps.tile([128, 512], F32, tag="ctx", name=f"ctx{sk}")
                          for sk in range(2)]
                ats = [None] * 8
                for step in range(8 + LOOKAHEAD):
                    tcp = step
                    if tcp < 8:
                        at = atp.tile([128, 2, 1024], F8, tag="at", name="at")
                        for i in range(2):
                            tcn = 2 * tcp + i
                            sc = scps.tile([128, 1024], F32, tag="sc", name="sc")
                            for sk in range(2):
                                nc.tensor.matmul(
                                    sc[:, sk * 512:(sk + 1) * 512],
                                    lhsT=kcat[h][:, tcn * 128:(tcn + 1) * 128],
                                    rhs=qcat[h][:, sk * 512:(sk + 1) * 512],
                                    start=True, stop=True)
                            emit_exp(at[:, i, :], sc[:])
                        ats[tcp] = at
                    # deferred two-stage evac of the previous head: rinv +
                    # broadcast early (psum long retired), muls 3 steps later
                    # (Pool result ready) so DVE is never head-of-line blocked
                    if step == 1 and prev_evac[0] is not None and attn_pv and attn_evac:
                        prev_rbs[0] = evac_stage1(h - 1, prev_evac[0])
                    if step == 4 and prev_evac[0] is not None:
                        if attn_pv and attn_evac:
                            evac_stage2(h - 1, prev_evac[0], prev_rbs[0])
                        prev_evac[0] = None
                    pv = step - LOOKAHEAD
                    if pv >= 0 and attn_pv:
                        at = ats[pv]
                        for sk in range(2):
                            nc.tensor.matmul(
                                ctx_ps[sk][:],
                                lhsT=vs4[:, 2 * (h * 8 + pv):2 * (h * 8 + pv) + 2, :],
                                rhs=at[:, :, sk * 512:(sk + 1) * 512],
                                start=(pv == 0), stop=(pv == 7),
                                perf_mode=PM.DoubleRow)
                prev_evac[0] = ctx_ps
            if attn_pv and attn_evac:
                ctx_evac(H - 1, prev_evac[0])
                if not evac_nd:
                    nc.sync.dma_start(ctxT2[64:128, :, :], co4[:])

        # output projection + residual + layernorm
        if not do_out:
            return
        with (
            tc.tile_pool(name="xs", bufs=2) as xsp,
            tc.tile_pool(name="ss", bufs=2) as ssp,
            tc.tile_pool(name="out_ps", bufs=4, space="PSUM") as ops,
        ):
            for st in range(8):
                po = ops.tile([128, 512], F32, tag="po")
                for pair in range(4):
                    nc.tensor.matmul(
                        po[:],
                        lhsT=ctxT2[:, pair, st * 128:(st + 1) * 128],
                        rhs=wo_sb[:, pair, :], start=(pair == 0), stop=(pair == 3))
                x = xsp.tile([128, D], F32, tag="x")
                nc.vector.tensor_add(x[:], po[:], resb[:, st, :])
                s1 = ssp.tile([128, 1], F32, tag="s1")
                nc.vector.tensor_reduce(s1[:], x[:], axis=Ax.X, op=Alu.add)
                mu = ssp.tile([128, 1], F32, tag="mu")
                nc.vector.tensor_scalar_mul(mu[:], s1[:], 1.0 / D)
                xc = xsp.tile([128, D], F32, tag="xc")
                nc.vector.tensor_scalar_sub(xc[:], x[:], mu[:])
                sq = xsp.tile([128, D], F32, tag="sq")
                ss = ssp.tile([128, 1], F32, tag="ss")
                nc.vector.scalar_tensor_tensor(
                    out=sq[:], in0=xc[:], scalar=1.0, in1=xc[:],
                    op0=Alu.bypass, op1=Alu.mult, accum_out=ss[:])
                var = ssp.tile([128, 1], F32, tag="var")
                nc.vector.tensor_scalar(
                    var[:], ss[:], 1.0 / D, EPS, op0=Alu.mult, op1=Alu.add)
                sd = ssp.tile([128, 1], F32, tag="sd")
                nc.scalar.sqrt(sd[:], var[:])
                rs = ssp.tile([128, 1], F32, tag="rs")
                nc.vector.reciprocal_approx_fast(rs[:], sd[:])
                y = xsp.tile([128, D], F32, tag="y")
                nc.vector.scalar_tensor_tensor(
                    out=y[:], in0=xc[:], scalar=rs[:], in1=gaB[:],
                    op0=Alu.mult, op1=Alu.mult)
                nc.vector.tensor_add(y[:], y[:], beB[:])
                nc.sync.dma_start(d_out.ap()[st * 128:(st + 1) * 128, :], y[:])


def get_module(reps=1):
    import os as _os
    phases = _os.environ.get("KPHASES", "all")
    key = (reps, phases)
    if key not in _MODULES:
        _MODULES[key] = _build_module(reps, phases)
    return _MODULES[key]


def make_in_maps(inputs):
    """Build the 8 per-core input maps from the full problem inputs."""
    w = {}
    for wn, key in (("w_qd", "Wq_d"), ("w_qt", "Wq_t"), ("w_kd", "Wk_d"),
                    ("w_kt", "Wk_t"), ("w_vd", "Wv_d"), ("w_vt", "Wv_t")):
        # [512 in, 512 out] -> [128 p, 4 kc, 512 out]
        w[wn] = np.ascontiguousarray(
            inputs[key].reshape(4, 128, HD).transpose(1, 0, 2)).astype(BF_NP)
    # Wo rows regrouped into head pairs: [128 = (h%2)*64+dh, 4 pairs, D]
    wo2 = np.ascontiguousarray(
        inputs["Wo"].reshape(4, 2, 64, D).transpose(1, 2, 0, 3).reshape(128, 4, D)
    ).astype(BF_NP)

    def bcol(v):
        # [512] -> [128 partition, 4 pair] so column p is the per-partition
        # bias for head-pair p's psum block
        return np.ascontiguousarray(v.reshape(4, 128).T).astype(np.float32)

    b = {
        "b_qd": bcol(inputs["bq_d"]),
        "b_qt": bcol(inputs["bq_t"]),
        "b_kd": bcol(inputs["bk_d"]),
        "b_kt": bcol(inputs["bk_t"]),
    }
    bv = (inputs["bv_d"].astype(np.float32)
          + inputs["bv_t"].astype(np.float32)).reshape(1, D)
    bo = inputs["bo"].reshape(1, D).astype(np.float32)
    gamma = inputs["gamma"].reshape(1, D).astype(np.float32)
    beta = inputs["beta"].reshape(1, D).astype(np.float32)

    kvT = {}
    for name, key in (("kdT", "K_data"), ("ktT", "K_time"),
                      ("vdT", "V_data"), ("vtT", "V_time")):
        kvT[name] = [
            np.ascontiguousarray(
                inputs[key][bb].astype(BF_NP).T) for bb in range(B)]

    in_maps = []
    for c in range(8):
        bb, half = divmod(c, 2)
        sl = slice(half * SQ, (half + 1) * SQ)
        m = {
            "qdT": np.ascontiguousarray(inputs["Q_data"][bb, sl, :].astype(BF_NP).T),
            "qtT": np.ascontiguousarray(inputs["Q_time"][bb, sl, :].astype(BF_NP).T),
            "kdT": kvT["kdT"][bb], "ktT": kvT["ktT"][bb],
            "vdT": kvT["vdT"][bb], "vtT": kvT["vtT"][bb],
            "qres": np.ascontiguousarray(inputs["Q_data"][bb, sl, :].astype(np.float32)),
            "wo2": wo2, "b_v": bv, "bo": bo, "gamma": gamma, "beta": beta,
        }
        m.update(w)
        m.update(b)
        in_maps.append(m)
    return in_maps


def kernel(**inputs):
    inputs = {k: np.asarray(v) for k, v in inputs.items()}
    nc = get_module(reps=1)
    in_maps = make_in_maps(inputs)
    res = bass_utils.run_bass_kernel_spmd(nc, in_maps, core_ids=list(range(8)))
    out = np.empty((B, S, D), dtype=np.float32)
    for c in range(8):
        bb, half = divmod(c, 2)
        out[bb, half * SQ:(half + 1) * SQ, :] = res.results[c]["out"]
    return out
